# revision 1
# baseline (speedup 1.0000x reference)
"""MLA attention (B=1, S=4096, d_model=1024, latent=512, H=16, D=64, causal+RoPE)
on 8 Trainium2 NeuronCores, tensor-parallel over heads (2 heads/core).

I/O-lean variant: each core uploads only its s-shard of x (512 rows), its
head-slice of wq/w_k_up/w_v_up/wo, a 1/8 latent-row shard of w_kv_down, and a
160KB RoPE factor table (full tables rebuilt on-device by angle addition:
table[p, 128q+r] = f(q-factor, r-factor) — exact, no Sin activation). On
device: x is transposed per-core on the PE, then x + w_kv_down are AllGathered
in ONE packed collective while DVE expands the RoPE tables; causal masks /
RoPE permutation / identity are generated with affine_select; weight slices
are transposed on the PE. The per-core output partials are ReduceScattered
on-device so each core downloads only its 512-row output shard. Total
host<->device traffic per call: ~62MB vs ~450MB for the replicate-everything
layout (host-side prep is views-only; output reassembly is zero-copy).

Compute core (unchanged from the validated baseline): feature-major fp32
storage / fp32r matmuls; RoPE via 32-row block-swap permutation matmul with a
sign-folded sin table; scores.T tiles via two 64-row-packed matmuls
(tile_position); P = exp(scores/8) with no max subtraction; PV with an
appended ones-column so the softmax denominator falls out of the matmul; late
1/l normalization fused into the per-head output projection combine.
"""

import numpy as np

S = 4096
DM = 1024
LAT = 512
H_PER_CORE = 2
D = 64
TW = 512           # s-tile width (moving free dim)
NEG = -1.0e30
NCORE = 8


def _host_factors(s_len):
    """RoPE table factors: table[p, 128q+r] = cos/sin((128q+r)*invf[p])
    expands by angle addition into products of [128,128] r-factors and
    [128, s_len/128] q-factors. Sign-fold (sin rows 0:32/64:96 negated)
    goes into BOTH sin factors; it cancels in the cos product."""
    NQ = s_len // 128
    inv = 1.0 / (10000.0 ** (np.arange(0, D, 2, dtype=np.float64) / D))
    invp = inv[np.arange(128) % 32]                       # [128]
    r = np.arange(128, dtype=np.float64)
    q = np.arange(NQ, dtype=np.float64) * 128.0
    CR = np.cos(invp[:, None] * r[None, :])
    SR = np.sin(invp[:, None] * r[None, :])
    CQ = np.cos(invp[:, None] * q[None, :])
    SQ = np.sin(invp[:, None] * q[None, :])
    fold = np.ones((128, 1))
    fold[0:32] = -1.0
    fold[64:96] = -1.0
    SR = SR * fold
    SQ = SQ * fold
    return np.ascontiguousarray(
        np.concatenate([CR, SR, CQ, SQ], axis=1)).astype(np.float32)


def build_program(s_len, reps=1):
    import concourse.bass as bass
    import concourse.bacc as bacc
    import concourse.tile as tile
    import concourse.mybir as mybir
    from contextlib import ExitStack

    f32 = mybir.dt.float32
    f32r = mybir.dt.float32r
    Exp = mybir.ActivationFunctionType.Exp
    mult = mybir.AluOpType.mult
    add = mybir.AluOpType.add
    is_ge = mybir.AluOpType.is_ge
    is_eq = mybir.AluOpType.is_equal

    SH = s_len // NCORE       # per-core s shard (512 at S=4096)
    SHC = SH // 128           # 128-row chunks per shard
    NT = s_len // TW          # number of 512-wide s tiles
    TT = s_len // 128         # number of 128-wide t tiles
    NK = TW // SH if TW > SH else 1   # gathered-x chunks per s-tile
    RG = [list(range(NCORE))]

    nc = bacc.Bacc("TRN2", target_bir_lowering=False, debug=False,
                   enable_asserts=False, num_devices=NCORE)

    x_sh = nc.dram_tensor("x_sh", [SH, DM], f32, kind="ExternalInput").ap()
    NQ = s_len // 128
    csf = nc.dram_tensor("csf", [128, 256 + 2 * NQ], f32,
                         kind="ExternalInput").ap()
    wq_sl = nc.dram_tensor("wq_sl", [128, DM], f32, kind="ExternalInput").ap()
    wkvd_sh = nc.dram_tensor("wkvd_sh", [LAT // NCORE, DM], f32,
                             kind="ExternalInput").ap()
    wkup_sl = nc.dram_tensor("wkup_sl", [128, LAT], f32,
                             kind="ExternalInput").ap()
    wvup_sl = nc.dram_tensor("wvup_sl", [128, LAT], f32,
                             kind="ExternalInput").ap()
    wo_sl = nc.dram_tensor("wo_sl", [DM, 128], f32, kind="ExternalInput").ap()
    out_sh = nc.dram_tensor("out_sh", [SH, DM], f32, kind="ExternalOutput").ap()

    def r(ap):
        return ap.bitcast(f32r)

    with tile.TileContext(nc) as tc:
        with ExitStack() as ctx:
            singles = ctx.enter_context(tc.tile_pool(name="singles", bufs=1))
            dram = ctx.enter_context(tc.tile_pool(name="dram", bufs=1,
                                                  space="DRAM"))

            wq_sb = singles.tile([128, DM], f32)           # chunk dc at dc*128
            wkvd_sb = singles.tile([128, 8 * LAT], f32)    # chunk dc at dc*512
            wkup_sb = singles.tile([128, LAT], f32)        # chunk lc at lc*128
            wvup_sb = singles.tile([128, LAT], f32)
            wo_sb = singles.tile([128, DM], f32)
            perm_sb = singles.tile([128, 128], f32)
            ident_sb = singles.tile([128, 128], f32)
            masks_sb = singles.tile([128, 4 * TW], f32)
            cos_sb = singles.tile([128, s_len], f32)
            sin_sb = singles.tile([128, s_len], f32)
            QR = singles.tile([128, s_len], f32)
            KR = singles.tile([128, s_len], f32)
            VR = singles.tile([128, TT * 130], f32)        # per t-tile: 64|1|64|1

            # one packed bounce per core: [xT shard | wkvd shard],
            # AllGathered in a single collective
            WB = DM // SH if DM > SH else 1     # wkvd flat rows per latent row
            WR = (64 * DM) // SH                # wkvd shard rows in the pack
            RP = DM + WR                        # pack rows
            pack = dram.tile([RP, SH], f32)
            gat = dram.tile([NCORE * RP, SH], f32, addr_space="Shared")
            osc = dram.tile([s_len, DM], f32)
            ored = dram.tile([SH, DM], f32)

            # ---------------- Phase A: constants + gathers + weight prep ----
            with ExitStack() as actx:
                asb = actx.enter_context(tc.tile_pool(name="asb", bufs=2))
                aps = actx.enter_context(
                    tc.tile_pool(name="apsum", bufs=2, space="PSUM"))

                ones = asb.tile([128, 128], f32, tag="ones")
                nc.vector.memset(ones[:], 1.0)
                nc.gpsimd.affine_select(ident_sb[:], ones[:], [[-1, 128]],
                                        is_eq, 0.0, base=0,
                                        channel_multiplier=1)
                # RoPE 32-row block-swap permutation, built in 4 column blocks
                for j0, b in ((0, -32), (32, 0), (64, -96), (96, -64)):
                    nc.gpsimd.affine_select(r(perm_sb[:, j0:j0 + 32]), ones[:, 0:32],
                                            [[-1, 32]], is_eq, 0.0, base=b,
                                            channel_multiplier=1)
                # causal masks: masks[p, r*TW+ss] = 0 if ss >= 128r+p else NEG
                zer = asb.tile([128, TW], f32, tag="zer")
                nc.vector.memset(zer[:], 0.0)
                for rr in range(4):
                    nc.gpsimd.affine_select(masks_sb[:, rr * TW:(rr + 1) * TW],
                                            zer[:], [[1, TW]], is_ge, NEG,
                                            base=-128 * rr,
                                            channel_multiplier=-1)
                # ones columns at 64/129 of each 130-wide V block
                vr3 = VR[:].rearrange("p (t c) -> p t c", c=130)
                nc.vector.memset(vr3[:, :, 64:65], 1.0)
                nc.vector.memset(vr3[:, :, 129:130], 1.0)

                # pack rows: [0:DM) xT shard, [DM:RP) wkvd shard
                # (row-major, flattened to SH cols)
                nc.sync.dma_start(
                    out=pack[DM:RP, :].rearrange("(a b) c -> a (b c)", b=WB),
                    in_=wkvd_sh)

                # x shard: load row-major, PE-transpose, write feature-major
                xt_sb = asb.tile([128, 8 * SH], f32, tag="xt")
                for sc in range(SHC):
                    xrow = asb.tile([128, DM], f32, tag="xrow")
                    nc.sync.dma_start(out=xrow[:],
                                      in_=x_sh[sc * 128:(sc + 1) * 128, :])
                    for dc in range(8):
                        pst = aps.tile([128, 128], f32, tag="tr")
                        nc.tensor.transpose(pst[:],
                                            xrow[:, dc * 128:(dc + 1) * 128],
                                            ident_sb[:])
                        nc.scalar.copy(
                            r(xt_sb[:, dc * SH + sc * 128:
                              dc * SH + (sc + 1) * 128]), pst[:])
                nc.sync.dma_start(
                    out=r(pack[0:DM, :]).rearrange("(dc p) c -> p dc c", dc=8),
                    in_=r(xt_sb[:]).rearrange("p (dc c) -> p dc c", dc=8))

                # single gather for x + tables + wkvd
                nc.gpsimd.collective_compute(
                    "AllGather", mybir.AluOpType.bypass, replica_groups=RG,
                    ins=[pack.opt()], outs=[gat.opt()])
                vxt = r(gat[:]).rearrange("(k q) c -> k q c", k=NCORE)

                # expand RoPE tables from factors (exact; overlaps the AG):
                # cos(128q+r) = CQ[q]*CR[r] - SQ[q]*SR[r]; sin analogous
                csf_sb = asb.tile([128, 256 + 2 * NQ], f32, tag="csf")
                nc.sync.dma_start(out=csf_sb[:], in_=csf)
                CRs = csf_sb[:, 0:128]
                SRs = csf_sb[:, 128:256]
                CQs = csf_sb[:, 256:256 + NQ]
                SQs = csf_sb[:, 256 + NQ:256 + 2 * NQ]
                sub = mybir.AluOpType.subtract
                for qq in range(NQ):
                    t1 = asb.tile([128, 128], f32, tag="csg1")
                    nc.vector.tensor_scalar_mul(t1[:], SRs, SQs[:, qq:qq + 1])
                    nc.vector.scalar_tensor_tensor(
                        out=cos_sb[:, qq * 128:(qq + 1) * 128], in0=CRs,
                        scalar=CQs[:, qq:qq + 1], in1=t1[:], op0=mult, op1=sub)
                    t2 = asb.tile([128, 128], f32, tag="csg2")
                    nc.vector.tensor_scalar_mul(t2[:], CRs, SQs[:, qq:qq + 1])
                    nc.vector.scalar_tensor_tensor(
                        out=sin_sb[:, qq * 128:(qq + 1) * 128], in0=SRs,
                        scalar=CQs[:, qq:qq + 1], in1=t2[:], op0=mult, op1=add)

                # weight slices: load row-major, PE-transpose into lhsT layouts
                def tr_chunks(dst, src_ap, n, dst_off):
                    """src rows [128, n*128]; dst cols chunk j at dst_off(j)."""
                    raw = asb.tile([128, n * 128], f32, tag=f"raw{n}")
                    nc.sync.dma_start(out=raw[:], in_=src_ap)
                    for j in range(n):
                        pst = aps.tile([128, 128], f32, tag="tr")
                        nc.tensor.transpose(pst[:],
                                            raw[:, j * 128:(j + 1) * 128],
                                            ident_sb[:])
                        nc.scalar.copy(r(dst[:, dst_off(j):dst_off(j) + 128]),
                                       pst[:])

                tr_chunks(wq_sb, wq_sl, 8, lambda j: j * 128)
                tr_chunks(wkup_sb, wkup_sl, 4, lambda j: j * 128)
                tr_chunks(wvup_sb, wvup_sl, 4, lambda j: j * 128)
                # wo_sl is [DM, 128] column-slice: transpose 128x128 blocks
                wo_raw = asb.tile([128, DM], f32, tag="woraw")
                nc.sync.dma_start(
                    out=r(wo_raw[:]).rearrange("p (dc c) -> p dc c", dc=8),
                    in_=r(wo_sl).rearrange("(dc p) c -> p dc c", dc=8))
                for dc in range(8):
                    pst = aps.tile([128, 128], f32, tag="tr")
                    nc.tensor.transpose(pst[:], wo_raw[:, dc * 128:(dc + 1) * 128],
                                        ident_sb[:])
                    nc.scalar.copy(r(wo_sb[:, dc * 128:(dc + 1) * 128]), pst[:])
                # wkvd: each gathered chunk k holds latent rows [64k, 64k+64)
                # row-major -> chunk (lc, dc) at cols dc*LAT + lc*128
                for lc in range(4):
                    kraw = asb.tile([128, DM], f32, tag="kraw")
                    for half in range(2):
                        src = gat[:].rearrange(
                            "(k q) c -> k q c",
                            k=NCORE)[2 * lc + half][DM:RP, :].rearrange(
                            "(a b) c -> a (b c)", b=WB)
                        nc.sync.dma_start(
                            out=kraw[half * 64:(half + 1) * 64, :], in_=src)
                    for dc in range(8):
                        pst = aps.tile([128, 128], f32, tag="tr")
                        nc.tensor.transpose(pst[:],
                                            kraw[:, dc * 128:(dc + 1) * 128],
                                            ident_sb[:])
                        nc.scalar.copy(
                            r(wkvd_sb[:, dc * LAT + lc * 128:
                              dc * LAT + (lc + 1) * 128]), pst[:])

            # ---------------- Stage B: projections + RoPE + V transpose ----
            for _rep in range(reps):
              with ExitStack() as bctx:
                  xpool = bctx.enter_context(tc.tile_pool(name="xpool", bufs=2))
                  latp = bctx.enter_context(tc.tile_pool(name="latp", bufs=2))
                  bp = bctx.enter_context(tc.tile_pool(name="bp", bufs=2))
                  projp = bctx.enter_context(
                      tc.tile_pool(name="projp", bufs=2, space="PSUM"))
                  trp = bctx.enter_context(
                      tc.tile_pool(name="trp", bufs=2, space="PSUM"))

                  for st in range(NT):
                      s0 = st * TW
                      k0 = (st * TW) // SH
                      xbig = xpool.tile([128, 8 * TW], f32, tag="xw")
                      for kk in range(NK):
                          nc.sync.dma_start(
                              out=r(xbig).rearrange("p (dc k c) -> p dc k c",
                                                    dc=8, k=NK)[:, :, kk, :],
                              in_=vxt[k0 + kk][0:DM, :].rearrange(
                                  "(dc p) c -> p dc c", dc=8))
                      xw = [xbig[:, dc * TW:(dc + 1) * TW] for dc in range(8)]

                      lat = []
                      for lc in range(4):
                          psl = projp.tile([128, TW], f32, tag="proj")
                          for dc in range(8):
                              nc.tensor.matmul(
                                  psl,
                                  lhsT=r(wkvd_sb[:, dc * LAT + lc * 128:
                                                 dc * LAT + (lc + 1) * 128]),
                                  rhs=r(xw[dc]),
                                  start=(dc == 0), stop=(dc == 7))
                          lt = latp.tile([128, TW], f32, tag=f"lat{lc}")
                          nc.scalar.copy(r(lt), psl)
                          lat.append(lt)

                      def rope(res, ps_raw, coff):
                          # ps_raw: PSUM tile with pre-rope projection
                          raw = bp.tile([128, TW], f32, tag=f"raw{coff}")
                          nc.vector.tensor_copy(r(raw), ps_raw)
                          pss = projp.tile([128, TW], f32, tag="proj")
                          nc.tensor.matmul(pss, lhsT=r(perm_sb[:]), rhs=r(raw),
                                           start=True, stop=True)
                          t1 = bp.tile([128, TW], f32, tag=f"ropetmp{coff}")
                          nc.vector.tensor_mul(t1, pss, sin_sb[:, s0:s0 + TW])
                          t2 = bp.tile([128, TW], f32, tag=f"ropetmp2{coff}")
                          nc.vector.tensor_mul(t2, raw, cos_sb[:, s0:s0 + TW])
                          nc.vector.tensor_add(r(res[:, s0:s0 + TW]), t2, t1)

                      psq = projp.tile([128, TW], f32, tag="proj")
                      for dc in range(8):
                          nc.tensor.matmul(
                              psq, lhsT=r(wq_sb[:, dc * 128:(dc + 1) * 128]),
                              rhs=r(xw[dc]), start=(dc == 0), stop=(dc == 7))
                      rope(QR, psq, "q")

                      psk = projp.tile([128, TW], f32, tag="proj")
                      for lc in range(4):
                          nc.tensor.matmul(
                              psk, lhsT=r(wkup_sb[:, lc * 128:(lc + 1) * 128]),
                              rhs=r(lat[lc]), start=(lc == 0), stop=(lc == 3))
                      rope(KR, psk, "k")

                      psv = projp.tile([128, TW], f32, tag="proj")
                      for lc in range(4):
                          nc.tensor.matmul(
                              psv, lhsT=r(wvup_sb[:, lc * 128:(lc + 1) * 128]),
                              rhs=r(lat[lc]), start=(lc == 0), stop=(lc == 3))
                      vt = bp.tile([128, TW], f32, tag="vt")
                      nc.scalar.copy(vt, psv)
                      for k4 in range(4):
                          pst = trp.tile([128, 128], f32, tag="tr")
                          nc.tensor.transpose(pst, vt[:, k4 * 128:(k4 + 1) * 128],
                                              ident_sb[:])
                          base = (st * 4 + k4) * 130
                          nc.scalar.copy(r(VR[:, base:base + 64]), pst[:, 0:64])
                          nc.scalar.copy(r(VR[:, base + 65:base + 129]),
                                         pst[:, 64:128])

                # ------------- Stage C: attention + output projection -------
              with ExitStack() as cctx:
                  spool = cctx.enter_context(
                      tc.tile_pool(name="spool", bufs=2, space="PSUM"))
                  opool = cctx.enter_context(
                      tc.tile_pool(name="opool", bufs=1, space="PSUM"))
                  wpool = cctx.enter_context(
                      tc.tile_pool(name="wpool", bufs=1, space="PSUM"))
                  ppool = cctx.enter_context(tc.tile_pool(name="ppool", bufs=3))
                  apool = cctx.enter_context(tc.tile_pool(name="apool", bufs=2))
                  lpool = cctx.enter_context(tc.tile_pool(name="lpool", bufs=2))
                  otpool = cctx.enter_context(tc.tile_pool(name="otpool", bufs=3))

                  for J in range(NT):
                      j0 = J * TW
                      ntt = 4 * (J + 1)
                      pso0 = opool.tile([65, TW], f32, tag="o0")
                      pso1 = opool.tile([65, TW], f32, tag="o1")
                      for tt in range(ntt):
                          t0 = tt * 128
                          pss0 = spool.tile([128, TW], f32, tag="s0")
                          pss1 = spool.tile([128, TW], f32, tag="s1")
                          nc.tensor.matmul(pss0,
                                           lhsT=r(KR[0:64, t0:t0 + 128]),
                                           rhs=r(QR[0:64, j0:j0 + TW]),
                                           start=True, stop=True,
                                           tile_position=(0, 0))
                          nc.tensor.matmul(pss1,
                                           lhsT=r(KR[64:128, t0:t0 + 128]),
                                           rhs=r(QR[64:128, j0:j0 + TW]),
                                           start=True, stop=True,
                                           tile_position=(64, 0))
                          dr = tt - 4 * J
                          if dr >= 0:  # diagonal tile: causal mask
                              m = masks_sb[:, dr * TW:(dr + 1) * TW]
                              nc.vector.tensor_add(pss0, pss0, m)
                              nc.vector.tensor_add(pss1, pss1, m)
                          p0 = ppool.tile([128, TW], f32, tag="p0")
                          p1 = ppool.tile([128, TW], f32, tag="p1")
                          nc.scalar.activation(r(p0), pss0, Exp, scale=0.125)
                          nc.scalar.activation(r(p1), pss1, Exp, scale=0.125)
                          vb = tt * 130
                          nc.tensor.matmul(pso0, lhsT=r(VR[:, vb:vb + 65]),
                                           rhs=r(p0),
                                           start=(tt == 0), stop=(tt == ntt - 1))
                          nc.tensor.matmul(pso1, lhsT=r(VR[:, vb + 65:vb + 130]),
                                           rhs=r(p1),
                                           start=(tt == 0), stop=(tt == ntt - 1))

                      at0 = apool.tile([65, TW], f32, tag="at0")
                      nc.scalar.copy(r(at0), pso0)
                      a1t = apool.tile([65, TW], f32, tag="a1t")
                      nc.scalar.copy(r(a1t), pso1)
                      at1 = apool.tile([128, TW], f32, tag="at1")
                      nc.sync.dma_start(out=r(at1[64:128, :]), in_=r(a1t[0:64, :]))

                      lt0 = lpool.tile([128, TW // 128], f32, tag="lt0")
                      lt1 = lpool.tile([128, TW // 128], f32, tag="lt1")
                      for j in range(TW // 128):
                          nc.sync.dma_start(
                              out=lt0[:, j:j + 1],
                              in_=at0[64:65, j * 128:(j + 1) * 128])
                          nc.sync.dma_start(
                              out=lt1[:, j:j + 1],
                              in_=a1t[64:65, j * 128:(j + 1) * 128])
                      li0 = lpool.tile([128, TW // 128], f32, tag="li0")
                      li1 = lpool.tile([128, TW // 128], f32, tag="li1")
                      nc.vector.reciprocal(li0, lt0)
                      nc.vector.reciprocal(li1, lt1)

                      for ss in range(TW // 128):
                          sg = j0 + ss * 128
                          for dh in range(2):
                              pw0 = wpool.tile([128, 512], f32, tag="w0")
                              pw1 = wpool.tile([128, 512], f32, tag="w1")
                              nc.tensor.matmul(
                                  pw0,
                                  lhsT=r(at0[0:64, ss * 128:(ss + 1) * 128]),
                                  rhs=r(wo_sb[0:64, dh * 512:(dh + 1) * 512]),
                                  start=True, stop=True, tile_position=(0, 0))
                              nc.tensor.matmul(
                                  pw1,
                                  lhsT=r(at1[64:128, ss * 128:(ss + 1) * 128]),
                                  rhs=r(wo_sb[64:128, dh * 512:(dh + 1) * 512]),
                                  start=True, stop=True, tile_position=(64, 0))
                              tmp = otpool.tile([128, 512], f32, tag="tmp")
                              nc.vector.tensor_scalar_mul(tmp, pw1,
                                                          li1[:, ss:ss + 1])
                              ot = otpool.tile([128, 512], f32, tag="ot")
                              nc.vector.scalar_tensor_tensor(
                                  out=ot, in0=pw0, scalar=li0[:, ss:ss + 1],
                                  in1=tmp, op0=mult, op1=add)
                              nc.sync.dma_start(
                                  out=osc[sg:sg + 128, dh * 512:(dh + 1) * 512],
                                  in_=ot)

              # on-device sum of per-core partials; each core keeps its shard
              nc.gpsimd.collective_compute(
                  "ReduceScatter", mybir.AluOpType.add, replica_groups=RG,
                  ins=[osc.opt()], outs=[ored.opt()])
            nc.sync.dma_start(out=out_sh, in_=ored[:])
    nc.compile()
    return nc


_CACHE = {}
_TABLES = {}


def _prep_inputs(x, wq, w_kv_down, w_k_up, w_v_up, wo, s_len):
    SH = s_len // NCORE
    LS = LAT // NCORE
    if s_len not in _TABLES:
        _TABLES[s_len] = _host_factors(s_len)
    csf = _TABLES[s_len]
    x2 = np.ascontiguousarray(x.reshape(s_len, DM)).astype(np.float32,
                                                           copy=False)
    in_maps = []
    for core in range(NCORE):
        sl = slice(core * 128, (core + 1) * 128)
        in_maps.append({
            "x_sh": x2[core * SH:(core + 1) * SH],
            "csf": csf,
            "wq_sl": wq[sl],
            "wkvd_sh": w_kv_down[core * LS:(core + 1) * LS],
            "wkup_sl": w_k_up[sl],
            "wvup_sl": w_v_up[sl],
            "wo_sl": wo[:, sl],
        })
    return in_maps


def kernel(x, wq, w_kv_down, w_k_up, w_v_up, wo):
    from concourse import bass_utils
    from concourse.bass_interp import get_hw_module

    s_len = x.shape[1]
    if s_len not in _CACHE:
        nc = build_program(s_len)
        nc.m = get_hw_module(nc.m)
        _CACHE[s_len] = nc
    nc = _CACHE[s_len]

    in_maps = _prep_inputs(np.asarray(x), np.asarray(wq), np.asarray(w_kv_down),
                           np.asarray(w_k_up), np.asarray(w_v_up),
                           np.asarray(wo), s_len)
    res = bass_utils.run_bass_kernel_spmd(nc, in_maps, core_ids=list(range(8)))
    shards = [res.results[c]["out_sh"] for c in range(NCORE)]
    base = shards[0].base
    if (base is not None and base.shape == (NCORE, s_len // NCORE, DM)
            and all(s.base is base for s in shards)):
        # run_bass_via_pjrt hands back views of one contiguous array in core
        # order; skip the 16MB reassembly copy
        return base.reshape(1, s_len, DM)
    out = np.concatenate(shards, axis=0)
    return out.reshape(1, s_len, DM)

